# revision 33
# baseline (speedup 1.0000x reference)
"""Trainium2 Bass kernel for ragged attention-pooling (EncoderTransformer).

Reference computation (per sample b, node n, position l):
    bags[n,l,:]  = seq[b, idx[n,l], :] * (l < len_n)
    pre          = bags @ W_pre + b_pre
    q_b          = hidden_flat[b] @ W_q
    energy[n,l]  = tanh(pre[n,l] + q_b) . v
    score        = softmax_l(energy)
    context[n]   = sum_l score[n,l] * bags[n,l]

Key reduction: energy[n,l] depends only on (b, s=idx[n,l]) for unmasked l and
is a per-sample constant for masked l.  With
    E[b,s]    = v . tanh(seq[b,s] @ W_pre + b_pre + q_b)
    emask[b]  = v . tanh(b_pre + q_b)
    C[s,n]    = #{l < len_n : idx[n,l] = s}        (count matrix)
    Ahat      = C * exp(E)[:,None]
we get exactly
    Z_n          = sum_s C[s,n] * expE[s] + (L - len_n) * exp(emask[b])
    context[n,:] = (sum_s Ahat[s,n] * seq[b,s,:]) / Z_n
so the whole kernel is dense.  The count matrix is built on-chip without any
indexed DMA (hardware scatter/gather primitives are Q7-descriptor-bound and
racy for duplicate rows): decompose s = g*128 + p and per node compute
    C_n[p, g] = sum_{l<len} 1[idx%128 = p] * 1[idx//128 = g]
as a tiny PE matmul A_n^T @ B_n of bf16 one-hot matrices built by DVE
iota-compares (masked l's get an out-of-range code so both one-hots vanish).
All count arithmetic is exact (0/1 products, integer counts, fp32 accum).

Sharding: data-parallel over samples: 8 cores x 2 samples (64 nodes) each.
Weights replicated.  Each core only touches its 2 samples' seq slice.
"""

import sys

if "/opt/trn_rl_repo" not in sys.path:
    sys.path.insert(0, "/opt/trn_rl_repo")

import numpy as np

B, S, D = 16, 1024, 512
NODE_NUM, L, DK = 32, 128, 64
NCORES = 8
BPC = B // NCORES            # samples per core = 2
NPC = BPC * NODE_NUM         # nodes per core = 64
CROWS = BPC * S              # count-matrix rows per core = 2048
NG = CROWS // 128            # s-chunks per core = 16
GPS = S // 128               # s-chunks per sample = 8
CW = NODE_NUM                # count-matrix width = 32

_CACHE = {}


def _build_nc(debug=False, loop_n=1, ablate=()):
    from concourse import bacc, mybir, tile
    from concourse.masks import make_identity

    f32 = mybir.dt.float32
    f32r = mybir.dt.float32r
    bf16 = mybir.dt.bfloat16
    alu = mybir.AluOpType
    act = mybir.ActivationFunctionType

    nc = bacc.Bacc("TRN2", target_bir_lowering=False, debug=False)

    seq = nc.dram_tensor("seq", [CROWS, D], f32, kind="ExternalInput")
    NBLOB = 714
    blob = nc.dram_tensor("blob", [128, NBLOB], f32, kind="ExternalInput")

    ctx_out = nc.dram_tensor("ctx_out", [NPC, D], f32, kind="ExternalOutput")
    if debug:
        dbg_c = nc.dram_tensor("dbg_c", [128, NG * CW], f32, kind="ExternalOutput")
        dbg_expE = nc.dram_tensor("dbg_expE", [128, NG], f32, kind="ExternalOutput")
        dbg_bias = nc.dram_tensor("dbg_bias", [DK, BPC], f32, kind="ExternalOutput")
        dbg_corr = nc.dram_tensor("dbg_corr", [1, NPC], f32, kind="ExternalOutput")

    import contextlib

    with tile.TileContext(nc) as tc:
        loop_ctx = tc.For_i(0, loop_n, 1) if loop_n > 1 else contextlib.nullcontext()
        with (
            loop_ctx,
            tc.tile_pool(name="seqp", bufs=1) as seqp,
            tc.tile_pool(name="seqtp", bufs=1) as seqtp,
            tc.tile_pool(name="wp", bufs=1) as wp,
            tc.tile_pool(name="smallp", bufs=1) as smallp,
            tc.tile_pool(name="csbp", bufs=1) as csbp,
            tc.tile_pool(name="ohp", bufs=6) as ohp,
            tc.tile_pool(name="outp", bufs=2) as outp,
            tc.tile_pool(name="ptr", bufs=2, space="PSUM") as ptr,
            tc.tile_pool(name="ppt", bufs=2, space="PSUM") as ppt,
            tc.tile_pool(name="pctx", bufs=2, space="PSUM") as pctx,
            tc.tile_pool(name="pcnt", bufs=1, space="PSUM") as pcnt,
            tc.tile_pool(name="psmall", bufs=1, space="PSUM") as psmall,
        ):
            # ---- constants / weights ----
            ident = wp.tile([128, 128], f32, tag="ident")
            make_identity(nc, ident[:])
            iota128 = wp.tile([128, 128], bf16, tag="iota128")
            nc.gpsimd.iota(
                iota128[:],
                pattern=[[1, 128]],
                base=0,
                channel_multiplier=0,
                allow_small_or_imprecise_dtypes=True,
            )
            iota8 = wp.tile([128, GPS], bf16, tag="iota8")
            nc.gpsimd.iota(
                iota8[:],
                pattern=[[1, GPS]],
                base=0,
                channel_multiplier=0,
                allow_small_or_imprecise_dtypes=True,
            )

            blob_t = wp.tile([128, NBLOB], f32, tag="blob")
            nc.sync.dma_start(out=blob_t[:], in_=blob.ap())
            wpre_t = blob_t
            wq_base, pmod_base, gdiv_base = 256, 512, 576
            hf_base, bpre_col, v_col, len_base = 640, 648, 649, 650
            wpre_r = wp.tile([128, 4 * DK], f32r, tag="wprer")
            nc.vector.tensor_copy(out=wpre_r[:], in_=blob_t[:, 0:256])
            wq_t = blob_t
            bpre_t = blob_t
            v_t = blob_t
            hf_t = blob_t
            lenr_t = blob_t
            pmod_t = blob_t
            gdiv_t = blob_t

            # ---- count matrix via one-hot matmuls (no indexed DMA) ----
            pct = pcnt.tile([128, NPC * GPS], f32, tag="pct", space="PSUM")
            for n in range(NPC if "counts" not in ablate else 0):
                a_oh = ohp.tile([L, 128], bf16, tag="a_oh", name=f"a{n}")
                nc.vector.tensor_scalar(
                    out=a_oh[:],
                    in0=iota128[0:L, :],
                    scalar1=pmod_t[:, pmod_base + n:pmod_base + n + 1],
                    scalar2=None,
                    op0=alu.is_equal,
                )
                b_oh = ohp.tile([L, GPS], bf16, tag="b_oh", name=f"b{n}")
                nc.vector.tensor_scalar(
                    out=b_oh[:],
                    in0=iota8[0:L, :],
                    scalar1=gdiv_t[:, gdiv_base + n:gdiv_base + n + 1],
                    scalar2=None,
                    op0=alu.is_equal,
                )
                nc.tensor.matmul(
                    out=pct[:, n * GPS:(n + 1) * GPS],
                    lhsT=a_oh[:],
                    rhs=b_oh[:],
                    start=True,
                    stop=True,
                )

            # reorder [p, (b, nn, g)] -> csb [p, (b, g, nn)]
            csb = csbp.tile([128, NG * CW], f32, tag="csb")
            if "counts" in ablate:
                nc.vector.memset(pct[:], 0.0)
            for b in range(BPC):
                nc.vector.tensor_copy(
                    out=csb[:, b * GPS * CW:(b + 1) * GPS * CW]
                    .rearrange("p (g n) -> p g n", n=CW),
                    in_=pct[:, b * CW * GPS:(b + 1) * CW * GPS]
                    .rearrange("p (n g) -> p n g", g=GPS)
                    .rearrange("p n g -> p g n"),
                )

            # ---- seq loads + on-chip transpose (seqT for the P matmul) ----
            sq4 = [
                seqp.tile([128, 4 * D], f32, tag=f"sq{i}", name=f"sq{i}")
                for i in range(NG // 4)
            ]
            for g in range(NG):
                nc.sync.dma_start(
                    out=sq4[g // 4][:, (g % 4) * D:(g % 4 + 1) * D],
                    in_=seq.ap()[g * 128:(g + 1) * 128, :],
                )

            def seq_sl(g, lo=0, hi=D):
                return sq4[g // 4][:, (g % 4) * D + lo:(g % 4) * D + hi]

            seqT = [
                seqtp.tile([128, CROWS], f32r, tag=f"seqT{k}", name=f"seqT{k}")
                for k in range(4)
            ]
            for k in range(4 if "transpose" not in ablate else 0):
                for g0 in range(0, NG, 4):
                    tr = ptr.tile([128, 512], f32, tag="tr", space="PSUM")
                    for j in range(4):
                        nc.tensor.transpose(
                            out=tr[:, j * 128:(j + 1) * 128],
                            in_=seq_sl(g0 + j, k * 128, (k + 1) * 128),
                            identity=ident[:],
                        )
                    nc.vector.tensor_copy(
                        out=seqT[k][:, g0 * 128:(g0 + 4) * 128], in_=tr[:]
                    )

            # ---- q = W_q^T @ hflat ; bias = b_pre + q ; emask path ----
            # psum_small: cols 0:2 q | 2:18 E^T | 18:20 emask | 20:22 Z
            psm = psmall.tile([128, 32], f32, tag="psm", space="PSUM")
            for k in range(4):
                nc.tensor.matmul(
                    out=psm[0:DK, 0:BPC],
                    lhsT=wq_t[:, wq_base + k * DK:wq_base + (k + 1) * DK],
                    rhs=hf_t[:, hf_base + k * BPC:hf_base + (k + 1) * BPC],
                    start=(k == 0),
                    stop=(k == 3),
                )
            bias_t = smallp.tile([DK, BPC], f32, tag="bias")
            nc.vector.tensor_scalar_add(bias_t[:], psm[0:DK, 0:BPC], bpre_t[0:DK, bpre_col:bpre_col + 1])
            tmask_t = smallp.tile([DK, BPC], f32, tag="tmask")
            nc.scalar.activation(tmask_t[:], bias_t[:], act.Tanh)
            nc.tensor.matmul(
                out=psm[0:1, 18:18 + BPC],
                lhsT=v_t[0:DK, v_col:v_col + 1],
                rhs=tmask_t[:],
                start=True,
                stop=True,
            )
            expmask_t = smallp.tile([1, BPC], f32, tag="expmask")
            nc.scalar.activation(expmask_t[:], psm[0:1, 18:18 + BPC], act.Exp)

            # corr_row[0, n] = (L - len_n); exp(emask) applied in the Z matmul
            corr_t = smallp.tile([1, NPC], f32, tag="corr")
            nc.vector.tensor_scalar(
                out=corr_t[:],
                in0=lenr_t[0:1, len_base:len_base + NPC],
                scalar1=-1.0,
                scalar2=float(L),
                op0=alu.mult,
                op1=alu.add,
            )
            # ---- P^T = W_pre^T @ seqT ; T = tanh(P^T + bias_b) ----
            T_t = smallp.tile([DK, CROWS], f32, tag="T")
            for s4 in range(4 if "ppath" not in ablate else 0):
                pt = ppt.tile([DK, 512], f32, tag="pt", space="PSUM")
                for k in range(4):
                    nc.tensor.matmul(
                        out=pt[:],
                        lhsT=wpre_r[:, k * DK:(k + 1) * DK],
                        rhs=seqT[k][:, s4 * 512:(s4 + 1) * 512],
                        start=(k == 0),
                        stop=(k == 3),
                    )
                b = s4 // 2
                nc.scalar.activation(
                    out=T_t[:, s4 * 512:(s4 + 1) * 512],
                    in_=pt[:],
                    func=act.Tanh,
                    bias=bias_t[:, b:b + 1],
                )

            # ---- E^T columns then expE [128, NG] ----
            for g in range(NG if "ppath" not in ablate else 0):
                nc.tensor.matmul(
                    out=psm[:, 2 + g:3 + g],
                    lhsT=T_t[:, g * 128:(g + 1) * 128],
                    rhs=v_t[0:DK, v_col:v_col + 1],
                    start=True,
                    stop=True,
                )
            expE_t = smallp.tile([128, NG], f32, tag="expE")
            nc.scalar.activation(expE_t[:], psm[:, 2:2 + NG], act.Exp)

            if debug:
                nc.sync.dma_start(out=dbg_c.ap(), in_=csb[:])
                nc.sync.dma_start(out=dbg_expE.ap(), in_=expE_t[:])
                nc.sync.dma_start(out=dbg_bias.ap(), in_=bias_t[:])
                nc.sync.dma_start(out=dbg_corr.ap(), in_=corr_t[:])

            # ---- Ahat = C * expE ----
            ahat = csbp.tile([128, NG * CW], f32, tag="ahat")
            for g in range(NG):
                nc.vector.tensor_scalar_mul(
                    ahat[:, g * CW:(g + 1) * CW],
                    csb[:, g * CW:(g + 1) * CW],
                    expE_t[:, g:g + 1],
                )

            # ---- context matmuls + Z + normalize ----
            for b in range(BPC if "ctx" not in ablate else 0):
                pc = pctx.tile([NODE_NUM, D], f32, tag="pc", space="PSUM")
                for k in range(GPS):
                    g = b * GPS + k
                    nc.tensor.matmul(
                        out=pc[:],
                        lhsT=ahat[:, g * CW:(g + 1) * CW],
                        rhs=seq_sl(g),
                        start=(k == 0),
                        stop=(k == GPS - 1),
                    )
                zcol = 20 + b
                for k in range(GPS):
                    g = b * GPS + k
                    nc.tensor.matmul(
                        out=psm[0:NODE_NUM, zcol:zcol + 1],
                        lhsT=csb[:, g * CW:(g + 1) * CW],
                        rhs=expE_t[:, g:g + 1],
                        start=(k == 0),
                        stop=False,
                    )
                nc.tensor.matmul(
                    out=psm[0:NODE_NUM, zcol:zcol + 1],
                    lhsT=corr_t[:, b * NODE_NUM:(b + 1) * NODE_NUM],
                    rhs=expmask_t[0:1, b:b + 1],
                    start=False,
                    stop=True,
                )
                rz = smallp.tile([NODE_NUM, 1], f32, tag=f"rz{b}", name=f"rz{b}")
                nc.vector.reciprocal(rz[:], psm[0:NODE_NUM, zcol:zcol + 1])
                octx = outp.tile([NODE_NUM, D], f32, tag=f"octx{b}", name=f"octx{b}")
                nc.vector.tensor_scalar_mul(octx[:], pc[:], rz[:])
                nc.sync.dma_start(
                    out=ctx_out.ap()[b * NODE_NUM:(b + 1) * NODE_NUM, :],
                    in_=octx[:],
                )

    nc.compile()
    return nc


def prep_in_maps(seq_output, hidden, index, lengths, W_pre, b_pre, W_q, v_att):
    """Host-side shard + index-metadata formatting (no tensor-data math)."""
    import ml_dtypes

    seq_output = np.ascontiguousarray(seq_output, dtype=np.float32)
    hidden = np.asarray(hidden, dtype=np.float32)
    index = np.asarray(index, dtype=np.int32)
    lengths = np.asarray(lengths, dtype=np.int32)
    hidden_flat = np.concatenate([hidden[0], hidden[1]], axis=1)  # [B, D]

    w_pre = np.ascontiguousarray(W_pre, dtype=np.float32)
    w_q = np.ascontiguousarray(W_q, dtype=np.float32)
    b_pre_c = np.ascontiguousarray(b_pre, dtype=np.float32).reshape(DK, 1)
    v_c = np.ascontiguousarray(v_att, dtype=np.float32).reshape(DK, 1)

    in_maps = []
    for c in range(NCORES):
        bs = slice(c * BPC, (c + 1) * BPC)
        seq_c = seq_output[bs].reshape(CROWS, D)
        hfT = np.ascontiguousarray(hidden_flat[bs].T)  # [D, BPC]
        idx_c = index[bs].astype(np.int64)  # [BPC, 32, L] in [0, S)
        len_c = lengths[c * NPC:(c + 1) * NPC].astype(np.int64)

        # [l, n_core] layouts of idx%128 / idx//128; masked -> out of range
        pm = (idx_c % 128).reshape(NPC, L).T.astype(np.float32)  # [L, NPC]
        gd = (idx_c // 128).reshape(NPC, L).T.astype(np.float32)
        maskT = np.arange(L)[:, None] < len_c[None, :]  # [L, NPC]
        pm[~maskT] = 200.0
        gd[~maskT] = 200.0

        blob = np.zeros((128, 714), np.float32)
        blob[:, 0:256] = w_pre.reshape(4, 128, DK).transpose(1, 0, 2).reshape(128, 256)
        blob[:, 256:512] = w_q.reshape(4, 128, DK).transpose(1, 0, 2).reshape(128, 256)
        blob[:, 512:576] = pm
        blob[:, 576:640] = gd
        blob[:, 640:648] = hfT.reshape(4, 128, BPC).transpose(1, 0, 2).reshape(128, 8)
        blob[0:DK, 648] = b_pre_c[:, 0]
        blob[0:DK, 649] = v_c[:, 0]
        blob[0, 650:650 + NPC] = len_c.astype(np.float32)

        in_maps.append(dict(seq=seq_c, blob=blob))
    return in_maps, hidden_flat


def get_nc(debug=False, loop_n=1, ablate=()):
    key = ("nc", debug, loop_n, tuple(ablate))
    if key not in _CACHE:
        _CACHE[key] = _build_nc(debug, loop_n, tuple(ablate))
    return _CACHE[key]


def kernel(seq_output, hidden, index, lengths, W_pre, b_pre, W_q, v_att):
    from concourse.bass_utils import run_bass_kernel_spmd

    nc = get_nc()
    in_maps, hidden_flat = prep_in_maps(
        seq_output, hidden, index, lengths, W_pre, b_pre, W_q, v_att
    )
    res = run_bass_kernel_spmd(nc, in_maps, list(range(NCORES)))
    nodes = np.concatenate(
        [res.results[c]["ctx_out"].reshape(BPC, NODE_NUM, D) for c in range(NCORES)],
        axis=0,
    )
    return nodes, hidden_flat


# revision 38
# speedup vs baseline: 1.9262x; 1.9262x over previous
"""Trainium2 Bass kernel for ragged attention-pooling (EncoderTransformer).

Reference computation (per sample b, node n, position l):
    bags[n,l,:]  = seq[b, idx[n,l], :] * (l < len_n)
    pre          = bags @ W_pre + b_pre
    q_b          = hidden_flat[b] @ W_q
    energy[n,l]  = tanh(pre[n,l] + q_b) . v
    score        = softmax_l(energy)
    context[n]   = sum_l score[n,l] * bags[n,l]

Key reduction: energy[n,l] depends only on (b, s=idx[n,l]) for unmasked l and
is a per-sample constant for masked l.  With
    E[b,s]    = v . tanh(seq[b,s] @ W_pre + b_pre + q_b)
    emask[b]  = v . tanh(b_pre + q_b)
    C[s,n]    = #{l < len_n : idx[n,l] = s}        (count matrix)
    Ahat      = C * exp(E)[:,None]
we get exactly
    Z_n          = sum_s C[s,n] * expE[s] + (L - len_n) * exp(emask[b])
    context[n,:] = (sum_s Ahat[s,n] * seq[b,s,:]) / Z_n
so the whole kernel is dense.  The count matrix is built on-chip without any
indexed DMA (hardware scatter/gather primitives are Q7-descriptor-bound and
racy for duplicate rows): decompose s = g*128 + p and per node compute
    C_n[p, g] = sum_{l<len} 1[idx%128 = p] * 1[idx//128 = g]
as a tiny PE matmul A_n^T @ B_n of bf16 one-hot matrices built by DVE
iota-compares (masked l's get an out-of-range code so both one-hots vanish).
All count arithmetic is exact (0/1 products, integer counts, fp32 accum).

Sharding: data-parallel over samples: 8 cores x 2 samples (64 nodes) each.
Weights replicated.  Each core only touches its 2 samples' seq slice.
"""

import sys

if "/opt/trn_rl_repo" not in sys.path:
    sys.path.insert(0, "/opt/trn_rl_repo")

import numpy as np

B, S, D = 16, 1024, 512
NODE_NUM, L, DK = 32, 128, 64
NCORES = 8
BPC = B // NCORES            # samples per core = 2
NPC = BPC * NODE_NUM         # nodes per core = 64
CROWS = BPC * S              # count-matrix rows per core = 2048
NG = CROWS // 128            # s-chunks per core = 16
GPS = S // 128               # s-chunks per sample = 8
CW = NODE_NUM                # count-matrix width = 32

_CACHE = {}


def _build_nc(debug=False, loop_n=1, ablate=()):
    from concourse import bacc, mybir, tile
    from concourse.masks import make_identity

    f32 = mybir.dt.float32
    f32r = mybir.dt.float32r
    bf16 = mybir.dt.bfloat16
    alu = mybir.AluOpType
    act = mybir.ActivationFunctionType

    nc = bacc.Bacc("TRN2", target_bir_lowering=False, debug=False)

    seq = nc.dram_tensor("seq", [CROWS, D], f32, kind="ExternalInput")
    NBLOB = 714
    blob = nc.dram_tensor("blob", [128, NBLOB], f32, kind="ExternalInput")

    ctx_out = nc.dram_tensor("ctx_out", [NPC, D], f32, kind="ExternalOutput")
    if debug:
        dbg_c = nc.dram_tensor("dbg_c", [128, NG * CW], f32, kind="ExternalOutput")
        dbg_expE = nc.dram_tensor("dbg_expE", [128, NG], f32, kind="ExternalOutput")
        dbg_bias = nc.dram_tensor("dbg_bias", [DK, BPC], f32, kind="ExternalOutput")
        dbg_corr = nc.dram_tensor("dbg_corr", [1, NPC], f32, kind="ExternalOutput")

    import contextlib

    with tile.TileContext(nc) as tc:
        loop_ctx = tc.For_i(0, loop_n, 1) if loop_n > 1 else contextlib.nullcontext()
        with (
            loop_ctx,
            tc.tile_pool(name="seqp", bufs=1) as seqp,
            tc.tile_pool(name="seqtp", bufs=1) as seqtp,
            tc.tile_pool(name="wp", bufs=1) as wp,
            tc.tile_pool(name="smallp", bufs=1) as smallp,
            tc.tile_pool(name="csbp", bufs=1) as csbp,
            tc.tile_pool(name="ohp", bufs=6) as ohp,
            tc.tile_pool(name="outp", bufs=2) as outp,
            tc.tile_pool(name="ptr", bufs=2, space="PSUM") as ptr,
            tc.tile_pool(name="ppt", bufs=2, space="PSUM") as ppt,
            tc.tile_pool(name="pctx", bufs=2, space="PSUM") as pctx,
            tc.tile_pool(name="pcnt", bufs=1, space="PSUM") as pcnt,
            tc.tile_pool(name="psmall", bufs=1, space="PSUM") as psmall,
        ):
            # ---- constants / weights ----
            ident = wp.tile([128, 128], f32, tag="ident")
            make_identity(nc, ident[:])
            iota128 = wp.tile([128, 128], bf16, tag="iota128")
            nc.gpsimd.iota(
                iota128[:],
                pattern=[[1, 128]],
                base=0,
                channel_multiplier=0,
                allow_small_or_imprecise_dtypes=True,
            )
            iota8 = wp.tile([128, GPS], bf16, tag="iota8")
            nc.gpsimd.iota(
                iota8[:],
                pattern=[[1, GPS]],
                base=0,
                channel_multiplier=0,
                allow_small_or_imprecise_dtypes=True,
            )

            blob_t = wp.tile([128, NBLOB], f32, tag="blob")
            nc.sync.dma_start(out=blob_t[:], in_=blob.ap())
            wpre_t = blob_t
            wq_base, pmod_base, gdiv_base = 256, 512, 576
            hf_base, bpre_col, v_col, len_base = 640, 648, 649, 650
            wpre_r = wp.tile([128, 4 * DK], f32r, tag="wprer")
            nc.vector.tensor_copy(out=wpre_r[:], in_=blob_t[:, 0:256])
            wq_t = blob_t
            bpre_t = blob_t
            v_t = blob_t
            hf_t = blob_t
            lenr_t = blob_t
            pmod_t = blob_t
            gdiv_t = blob_t

            # ---- count matrix via one-hot matmuls (no indexed DMA) ----
            pct = pcnt.tile([128, NPC * GPS], f32, tag="pct", space="PSUM")
            for n in range(NPC if "counts" not in ablate else 0):
                a_oh = ohp.tile([L, 128], bf16, tag="a_oh", name=f"a{n}")
                nc.vector.tensor_scalar(
                    out=a_oh[:],
                    in0=iota128[0:L, :],
                    scalar1=pmod_t[:, pmod_base + n:pmod_base + n + 1],
                    scalar2=None,
                    op0=alu.is_equal,
                )
                b_oh = ohp.tile([L, GPS], bf16, tag="b_oh", name=f"b{n}")
                nc.vector.tensor_scalar(
                    out=b_oh[:],
                    in0=iota8[0:L, :],
                    scalar1=gdiv_t[:, gdiv_base + n:gdiv_base + n + 1],
                    scalar2=None,
                    op0=alu.is_equal,
                )
                nc.tensor.matmul(
                    out=pct[:, n * GPS:(n + 1) * GPS],
                    lhsT=a_oh[:],
                    rhs=b_oh[:],
                    start=True,
                    stop=True,
                )

            # reorder [p, (b, nn, g)] -> csb [p, (b, g, nn)]
            csb = csbp.tile([128, NG * CW], f32, tag="csb")
            if "counts" in ablate:
                nc.vector.memset(pct[:], 0.0)
            for b in range(BPC):
                nc.vector.tensor_copy(
                    out=csb[:, b * GPS * CW:(b + 1) * GPS * CW]
                    .rearrange("p (g n) -> p g n", n=CW),
                    in_=pct[:, b * CW * GPS:(b + 1) * CW * GPS]
                    .rearrange("p (n g) -> p n g", g=GPS)
                    .rearrange("p n g -> p g n"),
                )

            # ---- seq loads + on-chip transpose (seqT for the P matmul) ----
            sq4 = [
                seqp.tile([128, 4 * D], f32, tag=f"sq{i}", name=f"sq{i}")
                for i in range(NG // 4)
            ]
            for g in range(NG):
                nc.sync.dma_start(
                    out=sq4[g // 4][:, (g % 4) * D:(g % 4 + 1) * D],
                    in_=seq.ap()[g * 128:(g + 1) * 128, :],
                )

            def seq_sl(g, lo=0, hi=D):
                return sq4[g // 4][:, (g % 4) * D + lo:(g % 4) * D + hi]

            seqT = [
                seqtp.tile([128, CROWS], f32r, tag=f"seqT{k}", name=f"seqT{k}")
                for k in range(4)
            ]

            # ---- q = W_q^T @ hflat ; bias = b_pre + q ; emask path ----
            # psum_small: cols 0:2 q | 2:18 E^T | 18:20 emask | 20:22 Z
            psm = psmall.tile([128, 32], f32, tag="psm", space="PSUM")
            for k in range(4):
                nc.tensor.matmul(
                    out=psm[0:DK, 0:BPC],
                    lhsT=wq_t[:, wq_base + k * DK:wq_base + (k + 1) * DK],
                    rhs=hf_t[:, hf_base + k * BPC:hf_base + (k + 1) * BPC],
                    start=(k == 0),
                    stop=(k == 3),
                )
            bias_t = smallp.tile([DK, BPC], f32, tag="bias")
            nc.vector.tensor_scalar_add(
                bias_t[:], psm[0:DK, 0:BPC], bpre_t[0:DK, bpre_col:bpre_col + 1]
            )
            tmask_t = smallp.tile([DK, BPC], f32, tag="tmask")
            nc.scalar.activation(tmask_t[:], bias_t[:], act.Tanh)
            nc.tensor.matmul(
                out=psm[0:1, 18:18 + BPC],
                lhsT=v_t[0:DK, v_col:v_col + 1],
                rhs=tmask_t[:],
                start=True,
                stop=True,
            )
            expmask_t = smallp.tile([1, BPC], f32, tag="expmask")
            nc.scalar.activation(expmask_t[:], psm[0:1, 18:18 + BPC], act.Exp)

            # corr_row[0, n] = (L - len_n); exp(emask) applied in the Z matmul
            corr_t = smallp.tile([1, NPC], f32, tag="corr")
            nc.vector.tensor_scalar(
                out=corr_t[:],
                in0=lenr_t[0:1, len_base:len_base + NPC],
                scalar1=-1.0,
                scalar2=float(L),
                op0=alu.mult,
                op1=alu.add,
            )

            # ---- fused per-s4 pipeline: transpose -> P -> tanh -> E ->
            #      exp -> scale -> ctx/Z partial accumulation ----
            T_t = smallp.tile([DK, CROWS], f32, tag="T")
            expE_t = smallp.tile([128, NG], f32, tag="expE")
            ahat = csbp.tile([128, NG * CW], f32, tag="ahat")
            pcs = {}
            zsbs = {}
            for s4 in range(4):
                b = s4 // 2
                g0 = s4 * 4
                for k in range(4):
                    tr = ptr.tile([128, 512], f32, tag="tr", space="PSUM")
                    for j in range(4):
                        nc.tensor.transpose(
                            out=tr[:, j * 128:(j + 1) * 128],
                            in_=seq_sl(g0 + j, k * 128, (k + 1) * 128),
                            identity=ident[:],
                        )
                    nc.scalar.activation(
                        out=seqT[k][:, g0 * 128:(g0 + 4) * 128],
                        in_=tr[:],
                        func=act.Copy,
                    )
                pt = ppt.tile([DK, 512], f32, tag="pt", space="PSUM")
                for k in range(4):
                    nc.tensor.matmul(
                        out=pt[:],
                        lhsT=wpre_r[:, k * DK:(k + 1) * DK],
                        rhs=seqT[k][:, s4 * 512:(s4 + 1) * 512],
                        start=(k == 0),
                        stop=(k == 3),
                    )
                nc.scalar.activation(
                    out=T_t[:, s4 * 512:(s4 + 1) * 512],
                    in_=pt[:],
                    func=act.Tanh,
                    bias=bias_t[:, b:b + 1],
                )
                for g in range(g0, g0 + 4):
                    nc.tensor.matmul(
                        out=psm[:, 2 + g:3 + g],
                        lhsT=T_t[:, g * 128:(g + 1) * 128],
                        rhs=v_t[0:DK, v_col:v_col + 1],
                        start=True,
                        stop=True,
                    )
                nc.scalar.activation(
                    expE_t[:, g0:g0 + 4], psm[:, 2 + g0:2 + g0 + 4], act.Exp
                )
                if b not in pcs:
                    pcs[b] = pctx.tile(
                        [NODE_NUM, D], f32, tag="pc", space="PSUM", name=f"pc{b}"
                    )
                    # init z accumulator with the masked-tail correction
                    nc.tensor.matmul(
                        out=psm[0:NODE_NUM, 20 + 2 * b:21 + 2 * b],
                        lhsT=corr_t[:, b * NODE_NUM:(b + 1) * NODE_NUM],
                        rhs=expmask_t[0:1, b:b + 1],
                        start=True,
                        stop=True,
                    )
                    zsb = smallp.tile(
                        [NODE_NUM, 1], f32, tag=f"zsb{b}", name=f"zsb{b}"
                    )
                    zsbs[b] = zsb
                    nc.vector.tensor_copy(
                        out=zsb[:], in_=psm[0:NODE_NUM, 20 + 2 * b:21 + 2 * b]
                    )
                for g in range(g0, g0 + 4):
                    nc.vector.tensor_scalar_mul(
                        ahat[:, g * CW:(g + 1) * CW],
                        csb[:, g * CW:(g + 1) * CW],
                        expE_t[:, g:g + 1],
                    )
                    nc.tensor.matmul(
                        out=pcs[b][:],
                        lhsT=ahat[:, g * CW:(g + 1) * CW],
                        rhs=seq_sl(g),
                        start=(g == b * GPS),
                        stop=(g == b * GPS + GPS - 1),
                    )
                    zcol = 20 + 2 * b + (g % 2)
                    nc.tensor.matmul(
                        out=psm[0:NODE_NUM, zcol:zcol + 1],
                        lhsT=csb[:, g * CW:(g + 1) * CW],
                        rhs=expE_t[:, g:g + 1],
                        start=True,
                        stop=True,
                    )
                    nc.vector.tensor_add(
                        zsbs[b][:],
                        zsbs[b][:],
                        psm[0:NODE_NUM, zcol:zcol + 1],
                    )
                if s4 % 2 == 1:
                    rz = smallp.tile(
                        [NODE_NUM, 1], f32, tag=f"rz{b}", name=f"rz{b}"
                    )
                    nc.vector.reciprocal(rz[:], zsbs[b][:])
                    octx = outp.tile(
                        [NODE_NUM, D], f32, tag=f"octx{b}", name=f"octx{b}"
                    )
                    nc.vector.tensor_scalar_mul(octx[:], pcs[b][:], rz[:])
                    nc.sync.dma_start(
                        out=ctx_out.ap()[b * NODE_NUM:(b + 1) * NODE_NUM, :],
                        in_=octx[:],
                    )

            if debug:
                nc.sync.dma_start(out=dbg_c.ap(), in_=csb[:])
                nc.sync.dma_start(out=dbg_expE.ap(), in_=expE_t[:])
                nc.sync.dma_start(out=dbg_bias.ap(), in_=bias_t[:])
                nc.sync.dma_start(out=dbg_corr.ap(), in_=corr_t[:])

    nc.compile()
    return nc


def prep_in_maps(seq_output, hidden, index, lengths, W_pre, b_pre, W_q, v_att):
    """Host-side shard + index-metadata formatting (no tensor-data math)."""
    import ml_dtypes

    seq_output = np.ascontiguousarray(seq_output, dtype=np.float32)
    hidden = np.asarray(hidden, dtype=np.float32)
    index = np.asarray(index, dtype=np.int32)
    lengths = np.asarray(lengths, dtype=np.int32)
    hidden_flat = np.concatenate([hidden[0], hidden[1]], axis=1)  # [B, D]

    w_pre = np.ascontiguousarray(W_pre, dtype=np.float32)
    w_q = np.ascontiguousarray(W_q, dtype=np.float32)
    b_pre_c = np.ascontiguousarray(b_pre, dtype=np.float32).reshape(DK, 1)
    v_c = np.ascontiguousarray(v_att, dtype=np.float32).reshape(DK, 1)

    in_maps = []
    for c in range(NCORES):
        bs = slice(c * BPC, (c + 1) * BPC)
        seq_c = seq_output[bs].reshape(CROWS, D)
        hfT = np.ascontiguousarray(hidden_flat[bs].T)  # [D, BPC]
        idx_c = index[bs].astype(np.int64)  # [BPC, 32, L] in [0, S)
        len_c = lengths[c * NPC:(c + 1) * NPC].astype(np.int64)

        # [l, n_core] layouts of idx%128 / idx//128; masked -> out of range
        pm = (idx_c % 128).reshape(NPC, L).T.astype(np.float32)  # [L, NPC]
        gd = (idx_c // 128).reshape(NPC, L).T.astype(np.float32)
        maskT = np.arange(L)[:, None] < len_c[None, :]  # [L, NPC]
        pm[~maskT] = 200.0
        gd[~maskT] = 200.0

        blob = np.zeros((128, 714), np.float32)
        blob[:, 0:256] = w_pre.reshape(4, 128, DK).transpose(1, 0, 2).reshape(128, 256)
        blob[:, 256:512] = w_q.reshape(4, 128, DK).transpose(1, 0, 2).reshape(128, 256)
        blob[:, 512:576] = pm
        blob[:, 576:640] = gd
        blob[:, 640:648] = hfT.reshape(4, 128, BPC).transpose(1, 0, 2).reshape(128, 8)
        blob[0:DK, 648] = b_pre_c[:, 0]
        blob[0:DK, 649] = v_c[:, 0]
        blob[0, 650:650 + NPC] = len_c.astype(np.float32)

        in_maps.append(dict(seq=seq_c, blob=blob))
    return in_maps, hidden_flat


def get_nc(debug=False, loop_n=1, ablate=()):
    key = ("nc", debug, loop_n, tuple(ablate))
    if key not in _CACHE:
        _CACHE[key] = _build_nc(debug, loop_n, tuple(ablate))
    return _CACHE[key]


def kernel(seq_output, hidden, index, lengths, W_pre, b_pre, W_q, v_att):
    from concourse.bass_utils import run_bass_kernel_spmd

    nc = get_nc()
    in_maps, hidden_flat = prep_in_maps(
        seq_output, hidden, index, lengths, W_pre, b_pre, W_q, v_att
    )
    res = run_bass_kernel_spmd(nc, in_maps, list(range(NCORES)))
    nodes = np.concatenate(
        [res.results[c]["ctx_out"].reshape(BPC, NODE_NUM, D) for c in range(NCORES)],
        axis=0,
    )
    return nodes, hidden_flat
